# revision 6
# baseline (speedup 1.0000x reference)
"""Farthest Point Sampling (FPS) Bass/TRN2 kernel, v6.

Problem: pos [16, 16384, 3] f32 -> indices [16*2048] int32 (exact FPS,
start index 0, ratio 1/8), bit-exact trajectory vs the f32 reference.

Sharding: batch 16 clouds -> 8 NeuronCores, 2 clouds per core (data
parallel). Each cloud is laid out as [128 partitions, 128 free]
(point n -> (n//128, n%128)).

Per FPS step per cloud:
  ACT : SQX/SQY/SQZ = Square(coord + bias)          bias = -c [128,1] AP
  DVE : t1 = sqx+sqy (stt); d = t1+sqz (stt); DIST = min(DIST, d) (stt)
  DVE : best6[:,0] = rowmax = reduce_max(DIST)
  DVE : maskR = is_eq(DIST, rowmax)                 per-partition argmax mask
  DVE : stt x4: best6[:,1+j] = sum_c maskR*MEGA_j   per-partition best x/y/z/gi
  PE  : b6T = transpose(best6[128,6]) -> PSUM [6,128]   (single-pass exact)
  DVE : m = reduce_max(b6T[0,:]);  maskrow = is_eq(b6T[0,:], m)  [1,128]
  PE  : onehotP = transpose(maskrow) -> PSUM [128,1]
  ACT : onehotP_sb = copy(onehotP)
  PE  : WB = matmul(onehotP_bcast[128,128], best6) -> PSUM [128,6]
        = winner row (m, x*, y*, z*, gi*) broadcast to all partitions
  DVE : bias = WB[:,1:4] * -1 -> biassb [128,3]
  ACT : outrow[0, s] = WB[0, 4:5]  (gi* as f32)
Host decodes idx = rint(outrow)."""

import numpy as np
from contextlib import ExitStack

import concourse.bass as bass
import concourse.mybir as mybir
from concourse.bass_utils import run_bass_kernel_spmd

AT = mybir.ActivationFunctionType
AL = mybir.AluOpType
AX = mybir.AxisListType
F32 = mybir.dt.float32

B, N, S = 16, 16384, 2048
N_CORES = 8
N_CLOUDS = 2  # per core
BIG = 1.0e10

_CACHE = {}
LABELS = {}


def _build_fps_kernel(S=S, n_clouds=N_CLOUDS):
    nc = bass.Bass(trn_type="TRN2", detect_race_conditions=False)
    mega_d = nc.dram_tensor("mega", [n_clouds, 128, 512], F32, kind="ExternalInput")
    bias0_d = nc.dram_tensor("bias0", [n_clouds, 128, 3], F32, kind="ExternalInput")
    ident_d = nc.dram_tensor("ident", [128, 128], F32, kind="ExternalInput")
    onesrow_d = nc.dram_tensor("onesrow", [1, 128], F32, kind="ExternalInput")
    out_d = nc.dram_tensor("outrow", [n_clouds, S], F32, kind="ExternalOutput")

    es = ExitStack()
    counter = [0]

    def sb(shape, dtype=F32):
        counter[0] += 1
        return es.enter_context(nc.sbuf_tensor(f"sb{counter[0]}", shape, dtype))

    def ps(shape, dtype=F32):
        counter[0] += 1
        return es.enter_context(nc.psum_tensor(f"ps{counter[0]}", shape, dtype))

    ident = sb([128, 128])
    onesrow = sb([1, 128])
    one11 = sb([1, 1])
    spc = sb([1, 1])
    junk_ps = ps([1, 1])

    cl = []
    for c in range(n_clouds):
        cl.append(dict(
            mega=sb([128, 512]),
            dist=sb([128, 128]),
            sqx=sb([128, 128]), sqy=sb([128, 128]), sqz=sb([128, 128]),
            t1=sb([128, 128]), dd=sb([128, 128]),
            maskR=sb([128, 128]),
            scr=sb([128, 128]),
            best6=sb([128, 6]),
            m_sb=sb([1, 1]),
            maskrow=sb([1, 128]),
            oneh=sb([128, 1]),
            biassb=sb([128, 3]),
            outrow=sb([1, S]),
            b6T_ps=ps([6, 128]),
            oneh_ps=ps([128, 1]),
            wb_ps=ps([128, 6]),
        ))

    sem_act = es.enter_context(nc.semaphore(name="sem_act"))
    sem_dve = es.enter_context(nc.semaphore(name="sem_dve"))
    sem_pe = es.enter_context(nc.semaphore(name="sem_pe"))
    sem_gp = es.enter_context(nc.semaphore(name="sem_gp"))

    sems = {"act": sem_act, "dve": sem_dve, "pe": sem_pe, "gp": sem_gp}
    engines = {"act": nc.scalar, "dve": nc.vector, "pe": nc.tensor, "gp": nc.gpsimd}
    count = {k: 0 for k in sems}
    waited = {(a, b): 0 for a in sems for b in sems}
    label = [None]

    def emit(eng, instr, inc=1):
        instr.then_inc(sems[eng], inc)
        count[eng] += inc
        if label[0] is not None:
            try:
                LABELS[instr.ins.name] = label[0]
            except Exception:
                pass
        return count[eng]

    def wait(consumer, producer, tick):
        if tick is None or consumer == producer:
            return
        if waited[(consumer, producer)] < tick:
            engines[consumer].wait_ge(sems[producer], tick)
            waited[(consumer, producer)] = tick

    for c in range(n_clouds):
        emit("gp", nc.gpsimd.dma_start(cl[c]["mega"][:], mega_d[c]), 16)
        emit("gp", nc.gpsimd.dma_start(cl[c]["biassb"][:], bias0_d[c]), 16)
    emit("gp", nc.gpsimd.dma_start(ident[:], ident_d[:]), 16)
    emit("gp", nc.gpsimd.dma_start(onesrow[:], onesrow_d[:]), 16)
    dma0 = count["gp"]
    wait("dve", "gp", dma0)
    emit("dve", nc.vector.memset(one11[:], 1.0))
    for c in range(n_clouds):
        emit("dve", nc.vector.memset(cl[c]["dist"][:], BIG))
        emit("dve", nc.vector.memset(cl[c]["outrow"][:], 0.0))
        emit("dve", nc.vector.memset(cl[c]["best6"][:], 0.0))
    wait("act", "gp", dma0)
    wait("pe", "gp", dma0)

    ticks = [dict() for _ in range(n_clouds)]

    # ---- phase functions -------------------------------------------------
    def head_act(c):
        """ACT: 3 squares (needs bias ready)."""
        t, tk = cl[c], ticks[c]
        label[0] = f"{'AB'[c]}.sq"
        if "bias" in tk:
            wait("act", "dve", tk["bias"])
        for j, sq in enumerate(("sqx", "sqy", "sqz")):
            tk[sq] = emit("act", nc.scalar.activation(
                t[sq][:], t["mega"][:, j * 128:(j + 1) * 128], AT.Square,
                bias=t["biassb"][:, j:j + 1], scale=1.0))

    def upd_a(c):
        """DVE: t1 = sqx+sqy."""
        t, tk = cl[c], ticks[c]
        label[0] = f"{'AB'[c]}.up"
        wait("dve", "act", tk["sqy"])
        tk["t1"] = emit("dve", nc.vector.scalar_tensor_tensor(
            t["t1"][:], t["sqx"][:], 1.0, t["sqy"][:], AL.mult, AL.add))

    def upd_b(c):
        """DVE: d = t1+sqz; DIST = min(DIST, d); rowmax."""
        t, tk = cl[c], ticks[c]
        label[0] = f"{'AB'[c]}.up"
        wait("dve", "act", tk["sqz"])
        tk["d"] = emit("dve", nc.vector.scalar_tensor_tensor(
            t["dd"][:], t["t1"][:], 1.0, t["sqz"][:], AL.mult, AL.add))
        tk["min"] = emit("dve", nc.vector.scalar_tensor_tensor(
            t["dist"][:], t["dd"][:], 1.0, t["dist"][:], AL.mult, AL.min))
        tk["rowmax"] = emit("dve", nc.vector.tensor_reduce(
            t["best6"][:, 0:1], t["dist"][:], axis=AX.X, op=AL.max))

    def gath(c):
        """DVE: maskR; stt x4 gather into best6[:,1:5]."""
        t, tk = cl[c], ticks[c]
        label[0] = f"{'AB'[c]}.ga"
        emit("dve", nc.vector.tensor_copy(spc[0:1, 0:1], one11[0:1, 0:1]))
        emit("dve", nc.vector.tensor_copy(spc[0:1, 0:1], one11[0:1, 0:1]))
        tk["maskR"] = emit("dve", nc.vector.tensor_tensor(
            t["maskR"][:], t["dist"][:], t["best6"][:, 0:1].broadcast_to((128, 128)), AL.is_equal))
        for j in range(4):
            tk["g"] = emit("dve", nc.vector.scalar_tensor_tensor(
                t["scr"][:], t["mega"][:, j * 128:(j + 1) * 128], 1.0, t["maskR"][:],
                AL.mult, AL.mult, accum_out=t["best6"][:, 1 + j:2 + j]))

    def tp6(c):
        """PE: transpose best6 -> [6,128] PSUM."""
        t, tk = cl[c], ticks[c]
        label[0] = f"{'AB'[c]}.tp"
        wait("pe", "dve", tk["g"])
        tk["tp6"] = emit("pe", nc.tensor.matmul(
            t["b6T_ps"][:], t["best6"][:], ident[:], start=True, stop=True))

    def midm(c):
        """DVE: m = max(b6T[0,:]); maskrow = is_eq."""
        t, tk = cl[c], ticks[c]
        label[0] = f"{'AB'[c]}.mm"
        wait("dve", "pe", tk["tp6"])
        tk["m"] = emit("dve", nc.vector.tensor_reduce(
            t["m_sb"][0:1, 0:1], t["b6T_ps"][0:1, :], axis=AX.X, op=AL.max))
        emit("dve", nc.vector.tensor_copy(spc[0:1, 0:1], one11[0:1, 0:1]))
        emit("dve", nc.vector.tensor_copy(spc[0:1, 0:1], one11[0:1, 0:1]))
        tk["mrow"] = emit("dve", nc.vector.tensor_tensor(
            t["maskrow"][0:1, :], t["b6T_ps"][0:1, :], t["m_sb"][0:1, 0:1].broadcast_to((1, 128)), AL.is_equal))

    def tpm(c):
        """PE: transpose maskrow -> onehotP PSUM [128,1]."""
        t, tk = cl[c], ticks[c]
        label[0] = f"{'AB'[c]}.tm"
        wait("pe", "dve", tk["mrow"])
        tk["tpm"] = emit("pe", nc.tensor.matmul(
            t["oneh_ps"][:], t["maskrow"][0:1, :], one11[0:1, 0:1], start=True, stop=True))

    def cpo(c):
        """ACT: copy onehotP PSUM -> SBUF."""
        t, tk = cl[c], ticks[c]
        label[0] = f"{'AB'[c]}.co"
        wait("act", "pe", tk["tpm"])
        tk["cpo"] = emit("act", nc.scalar.copy(t["oneh"][:], t["oneh_ps"][:]))

    def wbmm(c):
        """PE: WB = onehotP_bcast^T @ best6 -> [128,6] winner bcast."""
        t, tk = cl[c], ticks[c]
        label[0] = f"{'AB'[c]}.wb"
        wait("pe", "act", tk["cpo"])
        tk["wb"] = emit("pe", nc.tensor.matmul(
            t["wb_ps"][:], t["oneh"][:, 0:1].broadcast_to((128, 128)), t["best6"][:],
            start=True, stop=True))

    def tail(c, s):
        """DVE: bias = -WB[:,1:4]; ACT: outrow[0,s] = WB[0,4]."""
        t, tk = cl[c], ticks[c]
        label[0] = f"{'AB'[c]}.tl"
        wait("dve", "pe", tk["wb"])
        tk["bias"] = emit("dve", nc.vector.tensor_scalar(
            t["biassb"][:], t["wb_ps"][:, 1:4], -1.0, None, AL.mult))
        wait("act", "pe", tk["wb"])
        tk["out"] = emit("act", nc.scalar.copy(t["outrow"][0:1, s:s + 1], t["wb_ps"][0:1, 4:5]))

    # ---- schedule: software-pipelined, B half a step behind A ------------
    A, Bc = 0, 1

    def steady(s, first=False):
        # A runs step s head->tail; B finishes step s-1 tail, starts step s.
        head_act(A)
        if not first:
            gath(Bc)
            tp6(Bc)
        upd_a(A)
        upd_b(A)
        if not first:
            midm(Bc)
            tpm(Bc)
            cpo(Bc)
            wbmm(Bc)
        gath(A)
        if not first:
            tail(Bc, s - 1)
        tp6(A)
        if not first:
            head_act(Bc)
        midm(A)
        tpm(A)
        cpo(A)
        wbmm(A)
        upd_a(Bc)
        upd_b(Bc)
        tail(A, s)

    # prologue: step 1 for A then B (B's head needs its own bias path intact)
    head_act(A)
    upd_a(A)
    upd_b(A)
    gath(A)
    tp6(A)
    midm(A)
    tpm(A)
    cpo(A)
    wbmm(A)
    head_act(Bc)
    upd_a(Bc)
    upd_b(Bc)
    tail(A, 1)
    for s in range(2, S):
        steady(s)
    gath(Bc)
    tp6(Bc)
    midm(Bc)
    tpm(Bc)
    cpo(Bc)
    wbmm(Bc)
    tail(Bc, S - 1)

    for c in range(n_clouds):
        wait("gp", "act", ticks[c]["out"])
        wait("gp", "dve", ticks[c]["bias"])
        emit("gp", nc.gpsimd.dma_start(out_d[c], cl[c]["outrow"][0:1, :]), 16)

    es.close()
    return nc


def _make_inputs(pos_pair):
    ncl = pos_pair.shape[0]
    mega = np.empty((ncl, 128, 512), np.float32)
    bias0 = np.empty((ncl, 128, 3), np.float32)
    gi = np.arange(N, dtype=np.float32).reshape(128, 128)
    for c in range(ncl):
        for j in range(3):
            mega[c, :, j * 128:(j + 1) * 128] = pos_pair[c, :, j].reshape(128, 128)
        mega[c, :, 384:512] = gi
        bias0[c] = -pos_pair[c, 0]
    return {
        "mega": mega,
        "bias0": bias0,
        "ident": np.eye(128, dtype=np.float32),
        "onesrow": np.ones((1, 128), np.float32),
    }


def _get_nc():
    if "nc" not in _CACHE:
        _CACHE["nc"] = _build_fps_kernel()
    return _CACHE["nc"]


def run_on_cores(pos, **spmd_kwargs):
    """pos [16, 16384, 3] f32 -> (idx [16*2048] int32, BassKernelResults)."""
    pos = np.ascontiguousarray(np.asarray(pos, dtype=np.float32))
    assert pos.shape == (B, N, 3)
    nc = _get_nc()
    in_maps = [_make_inputs(pos[N_CLOUDS * c: N_CLOUDS * (c + 1)]) for c in range(N_CORES)]
    res = run_bass_kernel_spmd(nc, in_maps, core_ids=list(range(N_CORES)), **spmd_kwargs)
    idx = np.empty((B, S), np.int32)
    for core in range(N_CORES):
        outrow = res.results[core]["outrow"]  # [n_clouds, S]
        loc = np.rint(outrow).astype(np.int32)
        loc[:, 0] = 0
        for c in range(N_CLOUDS):
            b = N_CLOUDS * core + c
            idx[b] = loc[c] + b * N
    return idx.reshape(-1), res


def kernel(pos):
    idx, _ = run_on_cores(pos)
    return idx
